# revision 12
# baseline (speedup 1.0000x reference)
"""Trainium2 Bass kernel for conv3x3(valid) + bias + maxpool2x2 + relu.

Problem: x[64,1,512,512] f32, kernels[5,1,3,3], biases[5]
  -> out[64,5,255,255] f32.

Distribution: pure data parallel over 8 cores, 8 images per core.

Per-core algorithm:
- Conv as PE matmul in fp16 (host-rounded): contract dim K = 3
  column-shifted groups x 42 image rows, loaded in one 126-partition DMA
  per band via an overlapping-window access pattern. The 3 dj taps live
  in K via the shifted groups; the 3 di taps live in a banded lhsT
  within each group. Even/odd conv rows (parities E/O) go to separate
  PSUM banks at the same partition range; matmuls are split into even/odd
  column phases (stride-2 rhs views) so all pooling reductions become
  contiguous elementwise ops.
- Per superband-pair (2 bands x 20 pooled rows, M=100 = 5ch x 20 rows):
    ACT:  rE = relu(psE + bias)        -> fp16  (evacuates parity E)
    DVE:  H  = max(psO + bias, rE)     -> fp16  (scalar_tensor_tensor)
    DVE:  w  = max(H_q0, H_q1)         -> fp16  (W-pool, 2x packed mode)
    POOL: out DMA with fp16->fp32 cast (software DGE)
  Every ~3rd pair flips to an ACT-heavy variant (ACT evacuates both
  parities, DVE combines in SBUF at 2x) to balance ACT/DVE load.
"""

import numpy as np

N_CORES = 8
IMG_PER_CORE = 8
HW = 512
POOLED = 255
CONVW = 510

P_FULL = 20          # pooled rows per full superband
B_FULL = 12          # full superbands per image (12*20 + 15 = 255)
P_TAIL = 15
R_FULL = 42          # input rows per full superband (2*20+2)
R_TAIL = 32
IMGS_PER_CHUNK = 2
N_CHUNK = IMG_PER_CORE // IMGS_PER_CHUNK
BLK = B_FULL * IMGS_PER_CHUNK

_CACHE: dict = {}


def _build_lhsT(w: np.ndarray, parity: int, p_rows: int, in_rows: int) -> np.ndarray:
    """lhsT[g*in_rows + r, c*p_rows + io] = w[c, r-(2*io+parity), g]."""
    K = 3 * in_rows
    M = 5 * p_rows
    lhsT = np.zeros((K, M), dtype=np.float32)
    for g in range(3):
        for c in range(5):
            for io in range(p_rows):
                t = 2 * io + parity
                for dr in range(3):
                    r = t + dr
                    if r < in_rows:
                        lhsT[g * in_rows + r, c * p_rows + io] = w[c, dr, g]
    return lhsT


def _win_ap(x_ap, img, row0, nrows):
    """Source AP [[1,3],[512,nrows],[1,510]] at x[img, row0, 0]: enumerates
    (group-shift g, band row r, col w) with overlapping reads -- the three
    column-shifted K-groups of one band in a single 126-partition DMA."""
    import bass_rust

    c = x_ap.copy()
    c.offset = img * (HW * HW) + row0 * HW
    c.ap = bass_rust.VecI64Pair([[1, 3], [HW, nrows], [1, CONVW]])
    return c


def _build_program():
    import concourse.bacc as bacc
    import concourse.tile as tile
    from concourse import mybir
    from concourse.alu_op_type import AluOpType

    F32 = mybir.dt.float32
    F16 = mybir.dt.float16
    Relu = mybir.ActivationFunctionType.Relu

    nc = bacc.Bacc(trn_type="TRN2", target_bir_lowering=False, debug=False)

    x_ap = nc.dram_tensor("x", [IMG_PER_CORE, HW, HW], F16, kind="ExternalInput").ap()
    ltE = nc.dram_tensor("ltE", [126, 100], F16, kind="ExternalInput").ap()
    ltO = nc.dram_tensor("ltO", [126, 100], F16, kind="ExternalInput").ap()
    ltEt = nc.dram_tensor("ltEt", [96, 75], F16, kind="ExternalInput").ap()
    ltOt = nc.dram_tensor("ltOt", [96, 75], F16, kind="ExternalInput").ap()
    biasP = nc.dram_tensor("biasP", [100, 1], F32, kind="ExternalInput").ap()
    biasPt = nc.dram_tensor("biasPt", [75, 1], F32, kind="ExternalInput").ap()
    # Permuted output layouts (host unpermutes):
    # outA[c, io, i, pb, k, jo] = out[i, c, 20*(2*pb+k)+io, jo]
    # outB[c, io, i, jo]        = out[i, c, 240+io, jo]
    outA = nc.dram_tensor(
        "outA", [5, P_FULL, IMG_PER_CORE, B_FULL // 2, 2, POOLED], F32,
        kind="ExternalOutput",
    ).ap()
    outB = nc.dram_tensor(
        "outB", [5, P_TAIL, IMG_PER_CORE, POOLED], F32, kind="ExternalOutput"
    ).ap()

    def kqu(ap2d):
        """[M, 1024]-tile view [[256, 4],[1, 255]]: (bank*2+phase, u)."""
        return ap2d.rearrange("p (kq u) -> p kq u", kq=4)[:, :, 0:POOLED]

    with tile.TileContext(nc) as tc:
        with (
            tc.tile_pool(name="consts", bufs=1) as consts,
            tc.tile_pool(name="xc", bufs=2) as xpool,
            tc.tile_pool(name="xtail", bufs=1) as xtpool,
            tc.tile_pool(name="psE", bufs=2, space="PSUM") as psEp,
            tc.tile_pool(name="psO", bufs=2, space="PSUM") as psOp,
            tc.tile_pool(name="rE", bufs=3) as repool,
            tc.tile_pool(name="H", bufs=3) as hpool,
            tc.tile_pool(name="wt", bufs=4) as wpool,
        ):
            wE = consts.tile([126, 100], F16, tag="wE")
            wO = consts.tile([126, 100], F16, tag="wO")
            wEt = consts.tile([96, 75], F16, tag="wEt")
            wOt = consts.tile([96, 75], F16, tag="wOt")
            bt = consts.tile([100, 1], F32, tag="bt")
            btt = consts.tile([75, 1], F32, tag="btt")
            nc.sync.dma_start(wE[:], ltE[:])
            nc.sync.dma_start(wO[:], ltO[:])
            nc.sync.dma_start(wEt[:], ltEt[:])
            nc.sync.dma_start(wOt[:], ltOt[:])
            nc.sync.dma_start(bt[:], biasP[:])
            nc.sync.dma_start(btt[:], biasPt[:])

            Xt = xtpool.tile([96, HW * IMG_PER_CORE], F16, tag="Xt")
            state = {"pair": 0}

            def process_pair(rhs_cols, lt_e, lt_o, M, bias, out_dst):
                psE = psEp.tile([100, 1024], F32, tag="psE")
                psO = psOp.tile([100, 1024], F32, tag="psO")
                for wmat, ps in ((lt_e, psE), (lt_o, psO)):
                    K = wmat.shape[0]
                    for k, (xt, col) in enumerate(rhs_cols):
                        for q in (0, 1):
                            rhs = xt[0:K, col + q : col + q + CONVW : 2]
                            nc.tensor.matmul(
                                ps[0:M, 512 * k + 256 * q : 512 * k + 256 * q + 255],
                                wmat[:, :], rhs, start=True, stop=True,
                            )
                rE = repool.tile([100, 1024], F16, tag="rE")
                Ht = hpool.tile([100, 1024], F16, tag="H")
                heavy = state["pair"] % 4 == 2
                state["pair"] += 1
                nc.scalar.activation(
                    kqu(rE[0:M, :]), kqu(psE[0:M, :]), Relu,
                    bias=bias[0:M, :], scale=1.0,
                )
                if heavy:
                    rO = repool.tile([100, 1024], F16, tag="rO")
                    nc.scalar.activation(
                        kqu(rO[0:M, :]), kqu(psO[0:M, :]), Relu,
                        bias=bias[0:M, :], scale=1.0,
                    )
                    nc.vector.tensor_tensor(
                        kqu(Ht[0:M, :]), kqu(rE[0:M, :]), kqu(rO[0:M, :]),
                        op=AluOpType.max,
                    )
                else:
                    nc.vector.scalar_tensor_tensor(
                        kqu(Ht[0:M, :]), kqu(psO[0:M, :]), bias[0:M, :],
                        kqu(rE[0:M, :]), op0=AluOpType.add, op1=AluOpType.max,
                    )
                Hq = Ht[0:M, :].rearrange("p (k q u) -> p k q u", k=2, q=2)
                if state["pair"] % 2 == 1:
                    # W-pool folded into the output DMAs: write phase q0,
                    # then max-accumulate phase q1 (software DGE compute).
                    dst3 = out_dst.rearrange("p (k jo) -> p k jo", jo=POOLED)
                    nc.gpsimd.dma_start(dst3, Hq[:, :, 0, 0:POOLED])
                    nc.gpsimd.dma_start(
                        dst3, Hq[:, :, 1, 0:POOLED], accum_op=AluOpType.max
                    )
                else:
                    wt = wpool.tile([100, 512], F16, tag="wt")
                    wc = wt[0:M, 0:CONVW].rearrange("p (k u) -> p k u", u=POOLED)
                    nc.vector.tensor_tensor(
                        wc, Hq[:, :, 0, 0:POOLED], Hq[:, :, 1, 0:POOLED],
                        op=AluOpType.max,
                    )
                    nc.gpsimd.dma_start(out_dst, wt[0:M, 0:CONVW])

            for ch in range(N_CHUNK):
                i0 = ch * IMGS_PER_CHUNK
                X = xpool.tile([126, HW * BLK], F16, tag="X")
                for i in range(IMGS_PER_CHUNK):
                    for b in range(B_FULL):
                        col = HW * (i * B_FULL + b)
                        nc.sync.dma_start(
                            X[0:126, col : col + CONVW],
                            _win_ap(x_ap, i0 + i, 40 * b, R_FULL),
                        )
                if ch == 0:
                    for img in range(IMG_PER_CORE):
                        nc.sync.dma_start(
                            Xt[0:96, HW * img : HW * img + CONVW],
                            _win_ap(x_ap, img, 480, R_TAIL),
                        )

                for i in range(IMGS_PER_CHUNK):
                    for pb in range(B_FULL // 2):
                        cols = [
                            (X, HW * (i * B_FULL + 2 * pb)),
                            (X, HW * (i * B_FULL + 2 * pb + 1)),
                        ]
                        dst = outA[:, :, i0 + i, pb, :, :].rearrange(
                            "c io k jo -> (c io) (k jo)"
                        )
                        process_pair(cols, wE, wO, 100, bt, dst)

                # tail pair: images i0, i0+1 (one tail band each)
                tcols = [(Xt, HW * (i0 + 0)), (Xt, HW * (i0 + 1))]
                tdst = outB[:, :, i0 : i0 + 2, :].rearrange(
                    "c io i jo -> (c io) (i jo)"
                )
                process_pair(tcols, wEt, wOt, 75, btt, tdst)

    nc.compile()
    return nc


def _get_program():
    if "nc" not in _CACHE:
        _CACHE["nc"] = _build_program()
    return _CACHE["nc"]


def _host_inputs(kernels: np.ndarray, biases: np.ndarray):
    w = kernels.reshape(5, 3, 3).astype(np.float32)
    ltE = _build_lhsT(w, 0, P_FULL, R_FULL).astype(np.float16)
    ltO = _build_lhsT(w, 1, P_FULL, R_FULL).astype(np.float16)
    ltEt = _build_lhsT(w, 0, P_TAIL, R_TAIL).astype(np.float16)
    ltOt = _build_lhsT(w, 1, P_TAIL, R_TAIL).astype(np.float16)
    biasP = np.repeat(biases.astype(np.float32), P_FULL).reshape(100, 1)
    biasPt = np.repeat(biases.astype(np.float32), P_TAIL).reshape(75, 1)
    return ltE, ltO, ltEt, ltOt, biasP, biasPt


def kernel(x: np.ndarray, kernels: np.ndarray, biases: np.ndarray) -> np.ndarray:
    from concourse.bass_utils import run_bass_kernel_spmd

    nc = _get_program()
    ltE, ltO, ltEt, ltOt, biasP, biasPt = _host_inputs(
        np.asarray(kernels), np.asarray(biases)
    )
    xh = np.asarray(x, dtype=np.float32).reshape(64, HW, HW).astype(np.float16)

    in_maps = []
    for i in range(N_CORES):
        in_maps.append(
            {
                "x": xh[i * IMG_PER_CORE : (i + 1) * IMG_PER_CORE],
                "ltE": ltE,
                "ltO": ltO,
                "ltEt": ltEt,
                "ltOt": ltOt,
                "biasP": biasP,
                "biasPt": biasPt,
            }
        )
    res = run_bass_kernel_spmd(nc, in_maps, list(range(N_CORES)))
    out = np.empty((64, 5, POOLED, POOLED), dtype=np.float32)
    for i in range(N_CORES):
        a = res.results[i]["outA"].transpose(2, 0, 3, 4, 1, 5).reshape(
            IMG_PER_CORE, 5, P_FULL * B_FULL, POOLED
        )
        bpart = res.results[i]["outB"].transpose(2, 0, 1, 3)
        sl = slice(i * IMG_PER_CORE, (i + 1) * IMG_PER_CORE)
        out[sl, :, 0 : P_FULL * B_FULL, :] = a
        out[sl, :, P_FULL * B_FULL :, :] = bpart
    return out
